# revision 12
# baseline (speedup 1.0000x reference)
"""PixelCNN log_prob on 8 Trainium2 NeuronCores.

Data parallel over batch: 256 images -> 32 per core (16 image-pairs).
Masked convs are computed as tap-wise matmuls with 2 images packed
block-diagonally so the PE array sees K=128, M=128.

Layout per pair: SBUF tile [128 partitions, 66*66] bf16 holds the
zero-padded activations; partitions 0-63 = image A channels, 64-127 =
image B channels. Each conv tap is one matmul accumulating into PSUM
[128, 512] (one 8-row strip of the 64x64 image). ScalarE applies
bias+ReLU and writes the bf16 padded interior of the next layer buffer.

Final stage: logits for all 32 images are staged into a [32, 4096] fp32
tile (one image per partition); log_prob = sum(v*l) - sum(softplus(l))
via one fused DVE reduce + one fused ACT softplus-accumulate.
"""

import numpy as np
import ml_dtypes

N_CORES = 8
B = 256
PER_CORE = B // N_CORES          # 32 images
N_PAIRS = PER_CORE // 2          # 16
HD = 64                          # hidden channels
H = W = 64
HP = WP = 66                     # padded for 3x3
PIX = H * W                      # 4096
NSTRIP = 8
SW = PIX // NSTRIP               # 512
ROWS_PER_STRIP = H // NSTRIP     # 8

T3 = [(0, 0), (0, 1), (0, 2), (1, 0), (1, 1)]            # 3x3 mask-B taps
T7 = [(dy, dx) for dy in range(3) for dx in range(7)] + [(3, 0), (3, 1), (3, 2)]
N7 = len(T7)                     # 24

BF16 = ml_dtypes.bfloat16

_CACHE = {}


def _build_program():
    import concourse.bacc as bacc
    import concourse.tile as tile
    import concourse.mybir as mybir

    dt = mybir.dt
    AF = mybir.ActivationFunctionType
    ALU = mybir.AluOpType

    nc = bacc.Bacc("TRN2", target_bir_lowering=False, debug=False,
                   num_devices=N_CORES)

    xc_d = nc.dram_tensor("XC", [N_PAIRS, 2 * N7, PIX], dt.bfloat16,
                          kind="ExternalInput").ap()
    val_d = nc.dram_tensor("VAL", [PER_CORE, PIX], dt.float32,
                           kind="ExternalInput").ap()
    w0_d = nc.dram_tensor("W0C", [2 * N7, 128], dt.bfloat16,
                          kind="ExternalInput").ap()
    wc_d = nc.dram_tensor("WC", [128, 4 * 5 * 128], dt.bfloat16,
                          kind="ExternalInput").ap()
    wo_d = nc.dram_tensor("WOC", [128, 2], dt.bfloat16,
                          kind="ExternalInput").ap()
    b_d = nc.dram_tensor("BT", [128, 5], dt.float32,
                         kind="ExternalInput").ap()
    bo_d = nc.dram_tensor("BO32", [PER_CORE, 1], dt.float32,
                          kind="ExternalInput").ap()
    vc_d = nc.dram_tensor("VCORR", [PER_CORE, 1], dt.float32,
                          kind="ExternalInput").ap()
    out_d = nc.dram_tensor("OUT", [PER_CORE, 1], dt.float32,
                           kind="ExternalOutput").ap()

    with tile.TileContext(nc) as tc:
        with (
            tc.tile_pool(name="consts", bufs=1) as consts,
            tc.tile_pool(name="xc", bufs=2) as xcp,
            tc.tile_pool(name="lg", bufs=2) as lgp,
            tc.tile_pool(name="ps", bufs=6, space="PSUM") as psp,
            tc.tile_pool(name="pso", bufs=2, space="PSUM") as psop,
        ):
            w0_t = consts.tile([2 * N7, 128], dt.bfloat16)
            nc.sync.dma_start(w0_t[:], w0_d[:])
            wc_t = consts.tile([128, 4 * 5 * 128], dt.bfloat16)
            nc.sync.dma_start(wc_t[:], wc_d[:])
            wo_t = consts.tile([128, 2], dt.bfloat16)
            nc.sync.dma_start(wo_t[:], wo_d[:])
            b_t = consts.tile([128, 5], dt.float32)
            nc.sync.dma_start(b_t[:], b_d[:])
            bo_t = consts.tile([PER_CORE, 1], dt.float32)
            nc.sync.dma_start(bo_t[:], bo_d[:])
            vc_t = consts.tile([PER_CORE, 1], dt.float32)
            nc.sync.dma_start(vc_t[:], vc_d[:])
            val_t = consts.tile([PER_CORE, PIX], dt.float32)
            nc.sync.dma_start(val_t[:], val_d[:])
            logits_t = consts.tile([PER_CORE, PIX], dt.float32)
            scratch_t = consts.tile([PER_CORE, PIX], dt.float32)
            scratch2_t = consts.tile([PER_CORE, PIX], dt.float32)
            scratch3_t = consts.tile([PER_CORE, PIX], dt.float32)

            h_bufs = []
            for i in range(4):
                hb = consts.tile([128, HP * WP], dt.bfloat16, tag=f"h{i}")
                (nc.vector if i % 2 == 0 else nc.gpsimd).memset(hb[:], 0.0)
                h_bufs.append(hb)

            def hview(t):
                return t[:].rearrange("p (r c) -> p r c", c=WP)

            for p in range(N_PAIRS):
                xc_t = xcp.tile([2 * N7, PIX], dt.bfloat16)
                nc.sync.dma_start(xc_t[:], xc_d[p])

                ha = h_bufs[(2 * p) % 4]
                hb = h_bufs[(2 * p) % 4 + 1]

                # L0: 7x7 mask-A conv from host-im2col columns
                for s in range(NSTRIP):
                    ps = psp.tile([128, SW], dt.float32, tag="ps")
                    nc.tensor.matmul(ps[:], w0_t[:],
                                     xc_t[:, s * SW:(s + 1) * SW],
                                     start=True, stop=True)
                    nc.scalar.activation(
                        hview(ha)[:, 1 + s * 8:9 + s * 8, 1:65],
                        ps[:].rearrange("p (a b) -> p a b", b=W),
                        AF.Relu, bias=b_t[:, 0:1], scale=1.0)

                # L1..L4: 3x3 mask-B convs
                hin, hout = ha, hb
                for l in range(1, 5):
                    hv_in = hview(hin)
                    hv_out = hview(hout)
                    for s in range(NSTRIP):
                        ps = psp.tile([128, SW], dt.float32, tag="ps")
                        for ti, (dy, dx) in enumerate(T3):
                            nc.tensor.matmul(
                                ps[:],
                                wc_t[:, ((l - 1) * 5 + ti) * 128:
                                     ((l - 1) * 5 + ti + 1) * 128],
                                hv_in[:, s * 8 + dy:s * 8 + dy + 8,
                                      dx:dx + 64],
                                start=(ti == 0), stop=(ti == len(T3) - 1))
                        nc.scalar.activation(
                            hv_out[:, 1 + s * 8:9 + s * 8, 1:65],
                            ps[:].rearrange("p (a b) -> p a b", b=W),
                            AF.Relu, bias=b_t[:, l:l + 1], scale=1.0)
                    hin, hout = hout, hin

                # L5: 1x1 conv -> logits [2, PIX] staged per pair
                lg_t = lgp.tile([2, PIX], dt.float32)
                hv_in = hview(hin)
                for s in range(NSTRIP):
                    pso = psop.tile([2, SW], dt.float32, tag="pso")
                    nc.tensor.matmul(pso[:], wo_t[:],
                                     hv_in[:, 1 + s * 8:9 + s * 8, 1:65],
                                     start=True, stop=True)
                    nc.scalar.activation(lg_t[:, s * SW:(s + 1) * SW],
                                         pso[:], AF.Copy,
                                         bias=0.0, scale=1.0)
                nc.sync.dma_start(logits_t[2 * p:2 * p + 2, :], lg_t[:])

            # Final: lp = sum(v*l) - sum(softplus(l)) per image, with
            # l = raw + bo.  softplus(l) = relu(l) + ln(1 + exp(-|l|))
            # (all funcs live in the natural_log_exp_and_others table).
            n_sum = consts.tile([PER_CORE, 1], dt.float32)
            r_sum = consts.tile([PER_CORE, 1], dt.float32)
            sp_sum = consts.tile([PER_CORE, 1], dt.float32)
            vl_sum = consts.tile([PER_CORE, 1], dt.float32)
            res1_t = consts.tile([PER_CORE, 1], dt.float32)
            res_t = consts.tile([PER_CORE, 1], dt.float32)
            nc.scalar.activation(scratch_t[:], logits_t[:], AF.Abs,
                                 bias=bo_t[:], scale=1.0)
            nc.scalar.activation(scratch2_t[:], scratch_t[:], AF.Exp,
                                 bias=0.0, scale=-1.0)
            nc.scalar.activation(scratch_t[:], scratch2_t[:], AF.Ln,
                                 bias=1.0, scale=1.0, accum_out=n_sum[:])
            nc.scalar.activation(scratch2_t[:], logits_t[:], AF.Relu,
                                 bias=bo_t[:], scale=1.0, accum_out=r_sum[:])
            nc.vector.tensor_mul(scratch3_t[:], val_t[:], logits_t[:])
            nc.scalar.activation(scratch_t[:], scratch3_t[:], AF.Copy,
                                 bias=0.0, scale=1.0, accum_out=vl_sum[:])
            nc.vector.tensor_add(sp_sum[:], n_sum[:], r_sum[:])
            nc.vector.tensor_add(res1_t[:], vl_sum[:], vc_t[:])
            nc.vector.tensor_sub(res_t[:], res1_t[:], sp_sum[:])
            nc.sync.dma_start(out_d[:], res_t[:])

    nc.compile()
    return nc


def _prep_host(value, W0, b0, W1, b1, W2, b2, W3, b3, W4, b4, Wo, bo):
    # masks
    def mask(k, mtype):
        m = np.ones((k, k), np.float32)
        c = k // 2
        m[c, c + 1:] = 0.0
        m[c + 1:, :] = 0.0
        if mtype == 'A':
            m[c, c] = 0.0
        return m

    m7 = mask(7, 'A')
    m3 = mask(3, 'B')

    # W0C: [48, 128] block-diag im2col weights for the 7x7 mask-A conv
    W0m = W0 * m7[None, None]                      # [64,1,7,7]
    l0 = np.stack([W0m[:, 0, dy, dx] for (dy, dx) in T7], axis=0)  # [24,64]
    w0c = np.zeros((2 * N7, 128), np.float32)
    w0c[:N7, :64] = l0
    w0c[N7:, 64:] = l0

    # WC: [128, 4*5*128] per-(layer,tap) block-diag [cin, cout] weights
    wc = np.zeros((128, 4 * 5 * 128), np.float32)
    for li, Wl in enumerate((W1, W2, W3, W4)):
        Wlm = Wl * m3[None, None]
        for ti, (dy, dx) in enumerate(T3):
            sub = Wlm[:, :, dy, dx].T              # [cin, cout]
            col = (li * 5 + ti) * 128
            wc[0:64, col:col + 64] = sub
            wc[64:128, col + 64:col + 128] = sub

    # WOC: [128, 2]
    woc = np.zeros((128, 2), np.float32)
    woc[0:64, 0] = Wo[0, :, 0, 0]
    woc[64:128, 1] = Wo[0, :, 0, 0]

    # BT: [128, 5]; column l = concat(b_l, b_l)
    bt = np.stack([np.concatenate([bl, bl]) for bl in (b0, b1, b2, b3, b4)],
                  axis=1).astype(np.float32)
    bo32 = np.full((PER_CORE, 1), float(bo[0]), np.float32)

    # im2col for L0 over all images
    x = (value * 2.0 - 1.0).astype(np.float32)[:, 0]        # [256,64,64]
    xpad = np.pad(x, ((0, 0), (3, 3), (3, 3)))
    xcols = np.stack([xpad[:, dy:dy + 64, dx:dx + 64].reshape(B, PIX)
                      for (dy, dx) in T7], axis=1)          # [256,24,4096]
    xcols = xcols.astype(BF16)

    vals = value[:, 0].reshape(B, PIX).astype(np.float32)
    vcorr = (float(bo[0]) * vals.sum(axis=1, keepdims=True)).astype(np.float32)

    shared = {
        "W0C": w0c.astype(BF16),
        "WC": wc.astype(BF16),
        "WOC": woc.astype(BF16),
        "BT": bt,
        "BO32": bo32,
    }
    in_maps = []
    for c in range(N_CORES):
        lo = c * PER_CORE
        xc_core = xcols[lo:lo + PER_CORE].reshape(N_PAIRS, 2 * N7, PIX)
        in_maps.append({
            "XC": np.ascontiguousarray(xc_core),
            "VAL": np.ascontiguousarray(vals[lo:lo + PER_CORE]),
            "VCORR": np.ascontiguousarray(vcorr[lo:lo + PER_CORE]),
            **shared,
        })
    return in_maps


def kernel(**inputs):
    from concourse.bass_utils import run_bass_kernel_spmd

    if "nc" not in _CACHE:
        _CACHE["nc"] = _build_program()
    nc = _CACHE["nc"]

    in_maps = _prep_host(**inputs)
    res = run_bass_kernel_spmd(nc, in_maps, list(range(N_CORES)))
    out = np.concatenate([res.results[c]["OUT"][:, 0] for c in range(N_CORES)])
    return out.astype(np.float32)
